# revision 24
# baseline (speedup 1.0000x reference)
"""CommNet GNN message-passing kernel for 8 Trainium2 NeuronCores.

Computation (matches the jax reference):
    h = relu(x @ enc_w1 + enc_b1) @ enc_w2 + enc_b2
    for r in range(R):
        msg[i] = mean over edges (src==i) of h[dst]
        h = h + relu(msg @ comm_w[r] + comm_b[r])
    out = relu(h @ dec_w1 + dec_b1) @ dec_w2 + dec_b2

Sharding: nodes (and their outgoing edges, partitioned by source node id)
are sharded across the 8 cores; MLP weights replicated. Each comm round
the per-core h shards are AllGather'd (bf16) into a full copy of h in
each core's DRAM; per-edge h[dst] reads are serviced by dma_gather
(256B bf16 rows).

The kernel is SWDGE-bound: descriptor generation for the gathers costs
~8ns/edge on the GpSimd Q7 pair and dominates everything else. This
version therefore:
  - consolidates gathers into per-group instructions (4 tiles/group,
    2 gathers each) to amortize the ~1.2us/instruction fixed cost;
  - keeps h_full in bf16 (halves gather DMA + AllGather bytes);
  - splits each AllGather into 4 chunked collectives issued as soon as
    their tiles' h rows are written, hiding the collective under the
    previous round's gathers (only the last chunk is exposed);
  - trims the tail of each gather via trailing -1 indices (the Q7 ucode
    skips them), with memzero covering the resulting stale blocks.

Per-core compute layout ("T layout": features on partitions, nodes on the
free axis) so every MLP matmul chains without transposes; segment-mean
via one-hot matmul into PSUM as before.
"""

import numpy as np

N = 50000
D = 128
R = 2
NCORES = 8
TILES = 49                # src-node tiles of 128 per core
NL = TILES * 128          # 6272 nodes per core
NP = NCORES * NL          # 50176 padded node count
SPLIT = 32768             # int16 gather-index limit -> lo/hi split of h rows
CHUNK_TILES = (32, 17)    # AllGather chunking; chunk 0 = exactly SPLIT rows
GROUP = 3                 # tiles per consolidated gather
TRIM = False              # trailing -1 idx trim (per-core DGE reduction)

_PROGRAM_CACHE: dict = {}

TRACE = False
LAST_RESULTS = None


def _chunks():
    """[(chunk_start_tile, n_tiles, row_offset_in_h_full), ...]"""
    out = []
    t0 = 0
    row = 0
    for ct in CHUNK_TILES:
        out.append((t0, ct, row))
        t0 += ct
        row += NCORES * ct * D
    assert t0 == TILES and row == NP
    return out


def _groups():
    """Gather groups (lists of tile ids), in PROCESSING order.

    Chunk-1 tiles go first so the next round's h_hi AllGather (issued
    after chunk 1's last tile) fires early and its mesh latency hides
    under the remaining ~65% of the round.
    """
    gs = []
    for (t0, ct, _row) in reversed(_chunks()):
        t = t0
        while t < t0 + ct:
            g = list(range(t, min(t + GROUP, t0 + ct)))
            gs.append(g)
            t += GROUP
    return gs


# ----------------------------------------------------------------------------
# Device program
# ----------------------------------------------------------------------------

def build_program(layout):
    """Build the SPMD Bass program from the host-side layout dict."""
    import concourse.bass as bass
    import concourse.bacc as bacc
    import concourse.mybir as mybir
    import concourse.tile as tile

    dt = mybir.dt
    d = D
    nl = NL
    n_cores = NCORES
    tiles = TILES
    n_rounds = R

    groups = layout["groups"]      # list of tile lists
    lo_ba = layout["lo_ba"]        # per tile: first lo block in group section
    lo_bb = layout["lo_bb"]        # per tile: end lo block
    hi_ba = layout["hi_ba"]
    hi_bb = layout["hi_bb"]
    glo_nb = layout["glo_nb"]      # per group: total lo blocks
    ghi_nb = layout["ghi_nb"]      # per group: total hi blocks
    gilo_col = layout["gilo_col"]  # per group: col offset into ILO (int16 words)
    gihi_col = layout["gihi_col"]
    srcv_col = layout["srcv_col"]  # per tile: col offset into SRCV
    sum_b = layout["sum_b"]
    ilo_cols = layout["ilo_cols"]
    ihi_cols = layout["ihi_cols"]
    B_t = [(lo_bb[t] - lo_ba[t]) + (hi_bb[t] - hi_ba[t])
           for t in range(tiles)]
    gmax_nb = max(glo_nb[g] + ghi_nb[g] for g in range(len(groups)))
    bmax = max(B_t)

    nc = bacc.Bacc("TRN2", target_bir_lowering=False, debug=False,
                   num_devices=n_cores)

    # -------- kernel I/O --------
    xT_dram = nc.dram_tensor("xT", [d, nl], dt.float32, kind="ExternalInput")
    ilo_dram = nc.dram_tensor("ilo", [128, ilo_cols], dt.int16,
                              kind="ExternalInput")
    ihi_dram = nc.dram_tensor("ihi", [128, ihi_cols], dt.int16,
                              kind="ExternalInput")
    srcv_dram = nc.dram_tensor("srcv", [128, sum_b], dt.float32,
                               kind="ExternalInput")
    winv_dram = nc.dram_tensor("winv", [d, nl], dt.float32, kind="ExternalInput")
    iota_dram = nc.dram_tensor("iota", [d, d], dt.float32, kind="ExternalInput")
    ident_dram = nc.dram_tensor("ident", [d, d], dt.float32, kind="ExternalInput")
    w_drams = {}
    for wname in ("enc_w1", "enc_w2", "dec_w1", "dec_w2"):
        w_drams[wname] = nc.dram_tensor(wname, [d, d], dt.float32,
                                        kind="ExternalInput")
    for bname in ("enc_b1", "enc_b2", "dec_b1", "dec_b2"):
        w_drams[bname] = nc.dram_tensor(bname, [d, 1], dt.float32,
                                        kind="ExternalInput")
    for r in range(n_rounds):
        w_drams[f"cw{r}"] = nc.dram_tensor(f"cw{r}", [d, d], dt.float32,
                                           kind="ExternalInput")
        w_drams[f"cb{r}"] = nc.dram_tensor(f"cb{r}", [d, 1], dt.float32,
                                           kind="ExternalInput")
    outT_dram = nc.dram_tensor("outT", [d, nl], dt.float32,
                               kind="ExternalOutput")

    Relu = mybir.ActivationFunctionType.Relu
    Ident = mybir.ActivationFunctionType.Identity
    EQ = mybir.AluOpType.is_equal
    MUL = mybir.AluOpType.mult
    ADD = mybir.AluOpType.add

    with tile.TileContext(nc) as tc:
        with (
            tc.tile_pool(name="persist", bufs=1) as pp,
            tc.tile_pool(name="work", bufs=3) as wp,
            tc.tile_pool(name="gather", bufs=2) as gp,
            tc.tile_pool(name="ohpool", bufs=2) as op_,
            tc.tile_pool(name="ohfpool", bufs=1) as ofp,
            tc.tile_pool(name="psum", bufs=2, space="PSUM") as ps,
            tc.tile_pool(name="psum2", bufs=2, space="PSUM") as ps2,
            tc.tile_pool(name="dram", bufs=1, space="DRAM") as dp,
        ):
            # ---- persistent SBUF state ----
            xT = pp.tile([d, nl], dt.float32)
            hT = pp.tile([d, nl], dt.float32)
            winv = pp.tile([d, nl], dt.float32)
            iota = pp.tile([d, d], dt.float32)
            ident = pp.tile([d, d], dt.float32)
            wt = {}
            for wname in ("enc_w1", "enc_w2", "dec_w1", "dec_w2"):
                wt[wname] = pp.tile([d, d], dt.float32, tag=wname, name=wname)
            for bname in ("enc_b1", "enc_b2", "dec_b1", "dec_b2"):
                wt[bname] = pp.tile([d, 1], dt.float32, tag=bname, name=bname)
            for r in range(n_rounds):
                wt[f"cw{r}"] = pp.tile([d, d], dt.float32, tag=f"cw{r}",
                                       name=f"cw{r}")
                wt[f"cb{r}"] = pp.tile([d, 1], dt.float32, tag=f"cb{r}",
                                       name=f"cb{r}")

            nc.sync.dma_start(xT[:], xT_dram[:])
            nc.sync.dma_start(winv[:], winv_dram[:])
            nc.sync.dma_start(iota[:], iota_dram[:])
            nc.sync.dma_start(ident[:], ident_dram[:])
            for k, t in wt.items():
                nc.sync.dma_start(t[:], w_drams[k][:])

            # ---- DRAM scratch (bf16 h distribution) ----
            # Shared DRAM allows only one writer instruction per tensor, so
            # each AllGather chunk gets its own tensor; chunk 0 is exactly
            # the lo gather source, chunk 1 the hi source.
            ag_in = dp.tile([nl, d], dt.bfloat16)
            h_lo = [dp.tile([SPLIT, d], dt.bfloat16, addr_space="Shared",
                            tag=f"h_lo{r}", name=f"h_lo{r}")
                    for r in range(n_rounds)]
            h_hi = [dp.tile([NP - SPLIT, d], dt.bfloat16, addr_space="Shared",
                            tag=f"h_hi{r}", name=f"h_hi{r}")
                    for r in range(n_rounds)]

            def write_h_rows(t):
                """transpose hT[:, tile t] -> bf16 [node, feat] rows -> ag_in."""
                tsl = slice(t * d, (t + 1) * d)
                psT = ps2.tile([d, d], dt.float32, tag="psT")
                nc.tensor.transpose(psT[:], hT[:, tsl], ident[:])
                rowt = wp.tile([d, d], dt.bfloat16, tag="rowt")
                nc.scalar.copy(rowt[:], psT[:])
                nc.sync.dma_start(ag_in[tsl, :], rowt[:])

            def issue_ag(r, c):
                """AllGather chunk c of round r's h into h_lo/h_hi[r]."""
                t0, ct, _row = _chunks()[c]
                out = h_lo[r] if c == 0 else h_hi[r]
                nc.gpsimd.collective_compute(
                    "AllGather",
                    mybir.AluOpType.bypass,
                    replica_groups=[list(range(n_cores))],
                    ins=[ag_in[t0 * d:(t0 + ct) * d, :].opt()],
                    outs=[out.opt()],
                )

            # ================= encoder, chunked (chunk 1 first) ======
            for c, (t0, ct, _row) in reversed(list(enumerate(_chunks()))):
                t = t0
                while t < t0 + ct:
                    eg = min(4, t0 + ct - t)
                    gsl = slice(t * d, (t + eg) * d)
                    psA = ps.tile([d, 4 * d], dt.float32, tag="psA",
                                  padded_shape=[d, 4 * d])
                    nc.tensor.matmul(psA[:, 0:eg * d], wt["enc_w1"][:],
                                     xT[:, gsl], start=True, stop=True)
                    h1T = wp.tile([d, 4 * d], dt.float32, tag="h1T")
                    nc.scalar.activation(h1T[:, 0:eg * d], psA[:, 0:eg * d],
                                         Relu, bias=wt["enc_b1"][:])
                    psB = ps.tile([d, 4 * d], dt.float32, tag="psB",
                                  padded_shape=[d, 4 * d])
                    nc.tensor.matmul(psB[:, 0:eg * d], wt["enc_w2"][:],
                                     h1T[:, 0:eg * d], start=True, stop=True)
                    nc.scalar.activation(hT[:, gsl], psB[:, 0:eg * d], Ident,
                                         bias=wt["enc_b2"][:])
                    for tt in range(t, t + eg):
                        write_h_rows(tt)
                    t += eg
                issue_ag(0, c)

            # ================= comm rounds =================
            for r in range(n_rounds):
                src_lo = h_lo[r][:]
                src_hi = h_hi[r][:]
                chunk_done = {t0 + ct - 1: c for c, (t0, ct, _row)
                              in enumerate(_chunks())}

                gbufs = {}

                def alloc_gbuf(g):
                    gbufs[g] = gp.tile([128, gmax_nb, d], dt.bfloat16,
                                       tag="gbuf", name=f"gbuf_r{r}g{g}")

                def emit_lo(g):
                    n_lo = glo_nb[g] * 128
                    ilo = wp.tile([128, (gmax_nb * 128) // 16], dt.int16,
                                  tag="ilo")
                    nc.sync.dma_start(
                        ilo[:, 0:n_lo // 16],
                        ilo_dram[:, gilo_col[g]:gilo_col[g] + n_lo // 16])
                    nc.gpsimd.dma_gather(gbufs[g][:, 0:glo_nb[g], :], src_lo,
                                         ilo[:, 0:n_lo // 16], n_lo, n_lo,
                                         d, single_packet=False)

                def emit_hi(g):
                    lo_nb = glo_nb[g]
                    hi_nb = ghi_nb[g]
                    n_hi = hi_nb * 128
                    ihi = wp.tile([128, (gmax_nb * 128) // 16], dt.int16,
                                  tag="ihi")
                    nc.sync.dma_start(
                        ihi[:, 0:n_hi // 16],
                        ihi_dram[:, gihi_col[g]:gihi_col[g] + n_hi // 16])
                    nc.gpsimd.dma_gather(gbufs[g][:, lo_nb:lo_nb + hi_nb, :],
                                         src_hi, ihi[:, 0:n_hi // 16],
                                         n_hi, n_hi, d, single_packet=False)

                # Round starts with chunk-1 tiles, whose h_hi collective was
                # issued early in the previous round; the two frontloaded hi
                # gathers hide the h_lo collective's mesh latency.
                ng = len(groups)
                for g in range(min(2, ng)):
                    alloc_gbuf(g)
                    emit_hi(g)
                for g in range(min(2, ng)):
                    emit_lo(g)
                for g, gtiles in enumerate(groups):
                    lo_nb = glo_nb[g]
                    gbuf = gbufs[g]

                    for t in gtiles:
                        tsl = slice(t * d, (t + 1) * d)
                        bt = B_t[t]
                        # one-hot [128, bt*d]: oh[p, b*d + j] = (srcv[p,b] == j)
                        srcv = wp.tile([128, bmax], dt.float32, tag="srcv")
                        nc.sync.dma_start(
                            srcv[:, 0:bt],
                            srcv_dram[:, srcv_col[t]:srcv_col[t] + bt])
                        # DVE builds the one-hot in fp32 (a bf16 DVE output
                        # would engage 2-port mode and contend with the Q7's
                        # SBUF descriptor rings, stalling the gathers); the
                        # idle Scalar engine casts to bf16 for the matmul.
                        ohf = ofp.tile([128, bmax * d], dt.float32, tag="ohf")
                        oh = op_.tile([128, bmax * d], dt.bfloat16, tag="oh")
                        in0 = bass.AP(srcv.tensor, srcv.offset,
                                      [srcv.ap[0], [1, bt], [0, d]])
                        in1 = bass.AP(iota.tensor, iota.offset,
                                      [iota.ap[0], [0, bt], [1, d]])
                        out_oh = bass.AP(ohf.tensor, ohf.offset,
                                         [ohf.ap[0], [d, bt], [1, d]])
                        nc.vector.tensor_tensor(out_oh, in0, in1, EQ)
                        nc.scalar.copy(oh[:, 0:bt * d], ohf[:, 0:bt * d])
                        # segment sums: psM[f, n] += gathered_b.T @ onehot_b
                        psM = ps.tile([d, d], dt.float32, tag="psA")
                        nblk = 0
                        for b in range(lo_ba[t], lo_bb[t]):
                            nc.tensor.matmul(psM[:], gbuf[:, b, :],
                                             oh[:, nblk * d:(nblk + 1) * d],
                                             start=(nblk == 0),
                                             stop=(nblk == bt - 1))
                            nblk += 1
                        for b in range(hi_ba[t], hi_bb[t]):
                            nc.tensor.matmul(
                                psM[:], gbuf[:, lo_nb + b, :],
                                oh[:, nblk * d:(nblk + 1) * d],
                                start=(nblk == 0), stop=(nblk == bt - 1))
                            nblk += 1
                        # mean + comm MLP + residual
                        msgT = wp.tile([d, d], dt.float32, tag="msgT")
                        nc.vector.tensor_tensor(msgT[:], psM[:],
                                                winv[:, tsl], MUL)
                        psU = ps.tile([d, d], dt.float32, tag="psB")
                        nc.tensor.matmul(psU[:], wt[f"cw{r}"][:], msgT[:],
                                         start=True, stop=True)
                        updT = wp.tile([d, d], dt.float32, tag="updT")
                        nc.scalar.activation(updT[:], psU[:], Relu,
                                             bias=wt[f"cb{r}"][:])
                        nc.vector.tensor_tensor(hT[:, tsl], hT[:, tsl],
                                                updT[:], ADD)
                        if r + 1 < n_rounds:
                            write_h_rows(t)
                            if t in chunk_done:
                                issue_ag(r + 1, chunk_done[t])
                        else:
                            # final round: decoder fused per tile
                            psD = ps.tile([d, d], dt.float32, tag="psA",
                                          name="psD")
                            nc.tensor.matmul(psD[:], wt["dec_w1"][:],
                                             hT[:, tsl], start=True, stop=True)
                            d1T = wp.tile([d, d], dt.float32, tag="d1T")
                            nc.scalar.activation(d1T[:], psD[:], Relu,
                                                 bias=wt["dec_b1"][:])
                            psE = ps.tile([d, d], dt.float32, tag="psB",
                                          name="psE")
                            nc.tensor.matmul(psE[:], wt["dec_w2"][:], d1T[:],
                                             start=True, stop=True)
                            oT = wp.tile([d, d], dt.float32, tag="oT")
                            nc.scalar.activation(oT[:], psE[:], Ident,
                                                 bias=wt["dec_b2"][:])
                            nc.sync.dma_start(outT_dram[:, tsl], oT[:])

                    if g + 2 < ng:
                        alloc_gbuf(g + 2)
                        emit_hi(g + 2)
                        emit_lo(g + 2)

    nc.compile()
    return nc


# ----------------------------------------------------------------------------
# Host-side preparation
# ----------------------------------------------------------------------------

def _wrap_idx(idx):
    """int16 idx vector -> [128, n/16] layout: pos j -> (j%16, j//16), x8."""
    n = len(idx)
    a = np.zeros((16, n // 16), np.int16)
    a[np.arange(n) % 16, np.arange(n) // 16] = idx
    return np.tile(a, (8, 1))


def _row_remap():
    """node id (0..NP-1, old layout core-major) -> h_full row (chunk-major)."""
    remap = np.empty(NP, np.int64)
    n = np.arange(NP)
    k = n // NL
    t = (n % NL) // D
    p = n % D
    for (t0, ct, row) in _chunks():
        m = (t >= t0) & (t < t0 + ct)
        remap[n[m]] = (row + k[m] * ct * D + (t[m] - t0) * D + p[m])
    return remap


def host_prep(x, edge_index):
    """Shard + pad inputs; build per-core gather/one-hot side data."""
    d = D
    nl = NL
    n_real = x.shape[0]

    src = np.asarray(edge_index[0]).astype(np.int64)
    dst = np.asarray(edge_index[1]).astype(np.int64)

    cnt = np.bincount(src, minlength=NP).astype(np.float32)
    winv_full = 1.0 / np.maximum(cnt, 1.0)

    x_pad = np.zeros((NP, d), np.float32)
    x_pad[:n_real] = np.asarray(x, np.float32)

    remap = _row_remap()
    dstm = remap[dst]                # h_full row of each edge's dst

    # sort edges once by (tile, dst-row): tile-major grouping, ascending
    # dst row within a tile for friendlier gather locality
    tile_of_edge = src // d          # global tile id 0..n_cores*tiles-1
    order = np.lexsort((dstm, tile_of_edge))
    src_s, dstm_s = src[order], dstm[order]
    tile_s = tile_of_edge[order]
    lo_s = dstm_s < SPLIT

    n_tiles_g = NCORES * TILES
    tile_start = np.searchsorted(tile_s, np.arange(n_tiles_g))
    tile_end = np.searchsorted(tile_s, np.arange(n_tiles_g) + 1)
    n_lo_t = np.zeros(n_tiles_g, np.int64)
    for g in range(n_tiles_g):
        n_lo_t[g] = int(lo_s[tile_start[g]:tile_end[g]].sum())
    n_hi_t = (tile_end - tile_start) - n_lo_t

    # cross-core per-tile maxima (SPMD immediates) and minima (memzero range)
    core_ix = np.arange(NCORES) * TILES
    m_lo = [max(1, int(n_lo_t[core_ix + t].max())) for t in range(TILES)]
    m_hi = [max(1, int(n_hi_t[core_ix + t].max())) for t in range(TILES)]
    mn_lo = [int(n_lo_t[core_ix + t].min()) for t in range(TILES)]
    mn_hi = [int(n_hi_t[core_ix + t].min()) for t in range(TILES)]
    bl = [(m + d - 1) // d for m in m_lo]
    bh = [(m + d - 1) // d for m in m_hi]

    groups = _groups()
    # Unaligned slot packing: tile sections start at the running slot count
    # (not block-aligned); boundary blocks are shared between neighbouring
    # tiles and disambiguated by the -1 entries in each tile's srcv.
    lo_start = [0] * TILES    # slot offset of tile's lo section within group
    hi_start = [0] * TILES
    glo_nb, ghi_nb, gilo_col, gihi_col = [], [], [], []
    for g, gtiles in enumerate(groups):
        c = 0
        for t in gtiles:
            lo_start[t] = c
            c += m_lo[t]
        glo_nb.append((c + 127) // 128)
        c = 0
        for t in gtiles:
            hi_start[t] = c
            c += m_hi[t]
        ghi_nb.append((c + 127) // 128)
        gilo_col.append(sum(glo_nb[:g]) * 8)
        gihi_col.append(sum(ghi_nb[:g]) * 8)
    gilo_col = [0] * len(groups)
    gihi_col = [0] * len(groups)
    col_lo = col_hi = 0
    for g in range(len(groups)):
        gilo_col[g] = col_lo
        gihi_col[g] = col_hi
        col_lo += glo_nb[g] * 8
        col_hi += ghi_nb[g] * 8

    # per-tile block ranges [bA, bB) within the group's lo / hi sections
    lo_ba = [lo_start[t] // 128 for t in range(TILES)]
    lo_bb = [(lo_start[t] + m_lo[t] + 127) // 128 for t in range(TILES)]
    hi_ba = [hi_start[t] // 128 for t in range(TILES)]
    hi_bb = [(hi_start[t] + m_hi[t] + 127) // 128 for t in range(TILES)]

    srcv_col = [0] * TILES
    sb = 0
    for t in range(TILES):
        srcv_col[t] = sb
        sb += (lo_bb[t] - lo_ba[t]) + (hi_bb[t] - hi_ba[t])

    layout = {
        "m_lo": m_lo, "m_hi": m_hi, "groups": groups,
        "lo_start": lo_start, "hi_start": hi_start,
        "lo_ba": lo_ba, "lo_bb": lo_bb, "hi_ba": hi_ba, "hi_bb": hi_bb,
        "glo_nb": glo_nb, "ghi_nb": ghi_nb,
        "gilo_col": gilo_col, "gihi_col": gihi_col,
        "srcv_col": srcv_col,
        "sum_b": sb, "ilo_cols": col_lo, "ihi_cols": col_hi,
    }

    per_core = []
    for k in range(NCORES):
        ilo_all = np.zeros((128, col_lo), np.int16)
        ihi_all = np.zeros((128, col_hi), np.int16)
        srcv_all = np.full((128, sb), -1.0, np.float32)
        for g, gtiles in enumerate(groups):
            idx_lo = np.zeros(glo_nb[g] * 128, np.int16)
            idx_hi = np.zeros(ghi_nb[g] * 128, np.int16)
            for t in gtiles:
                gt = k * TILES + t
                s0, s1 = tile_start[gt], tile_end[gt]
                e_lo = np.flatnonzero(lo_s[s0:s1]) + s0
                e_hi = np.flatnonzero(~lo_s[s0:s1]) + s0
                idx_lo[lo_start[t]:lo_start[t] + len(e_lo)] = dstm_s[e_lo]
                idx_hi[hi_start[t]:hi_start[t] + len(e_hi)] = \
                    dstm_s[e_hi] - SPLIT
                # srcv covers the tile's block windows; -1 masks both the
                # pad slots and the neighbouring tiles' slots in shared
                # boundary blocks. slot s -> partition s%128, block s//128.
                nbl = lo_bb[t] - lo_ba[t]
                nbh = hi_bb[t] - hi_ba[t]
                bt = nbl + nbh
                slot_src = np.full(bt * 128, -1.0, np.float32)
                rel = lo_start[t] - lo_ba[t] * 128
                slot_src[rel:rel + len(e_lo)] = \
                    (src_s[e_lo] - gt * d).astype(np.float32)
                rel = nbl * 128 + (hi_start[t] - hi_ba[t] * 128)
                slot_src[rel:rel + len(e_hi)] = \
                    (src_s[e_hi] - gt * d).astype(np.float32)
                srcv_all[:, srcv_col[t]:srcv_col[t] + bt] = \
                    slot_src.reshape(bt, 128).T
            ilo_all[:, gilo_col[g]:gilo_col[g] + glo_nb[g] * 8] = \
                _wrap_idx(idx_lo)
            ihi_all[:, gihi_col[g]:gihi_col[g] + ghi_nb[g] * 8] = \
                _wrap_idx(idx_hi)
        ksl = slice(k * nl, (k + 1) * nl)
        per_core.append({
            "xT": np.ascontiguousarray(x_pad[ksl].T),
            "ilo": ilo_all,
            "ihi": ihi_all,
            "srcv": srcv_all,
            "winv": np.ascontiguousarray(
                np.tile(winv_full[ksl][None, :], (d, 1))),
        })
    return per_core, layout


def kernel(x, edge_index, enc_w1, enc_b1, enc_w2, enc_b2,
           comm_w, comm_b, dec_w1, dec_b1, dec_w2, dec_b2):
    from concourse.bass_utils import run_bass_kernel_spmd

    x = np.asarray(x)
    n_real = x.shape[0]
    per_core, layout = host_prep(x, np.asarray(edge_index))

    key = (tuple(layout["m_lo"]), tuple(layout["m_hi"]))
    if key not in _PROGRAM_CACHE:
        _PROGRAM_CACHE[key] = build_program(layout)
    nc = _PROGRAM_CACHE[key]

    iota_np = np.tile(np.arange(D, dtype=np.float32)[None, :], (D, 1))
    ident_np = np.eye(D, dtype=np.float32)
    shared = {
        "iota": iota_np,
        "ident": ident_np,
        "enc_w1": np.asarray(enc_w1, np.float32),
        "enc_w2": np.asarray(enc_w2, np.float32),
        "dec_w1": np.asarray(dec_w1, np.float32),
        "dec_w2": np.asarray(dec_w2, np.float32),
        "enc_b1": np.asarray(enc_b1, np.float32).reshape(D, 1),
        "enc_b2": np.asarray(enc_b2, np.float32).reshape(D, 1),
        "dec_b1": np.asarray(dec_b1, np.float32).reshape(D, 1),
        "dec_b2": np.asarray(dec_b2, np.float32).reshape(D, 1),
    }
    for r in range(R):
        shared[f"cw{r}"] = np.asarray(comm_w[r], np.float32)
        shared[f"cb{r}"] = np.asarray(comm_b[r], np.float32).reshape(D, 1)

    in_maps = [{**shared, **pc} for pc in per_core]
    res = run_bass_kernel_spmd(nc, in_maps, core_ids=list(range(NCORES)),
                               trace=TRACE)
    global LAST_RESULTS
    LAST_RESULTS = res

    out = np.empty((NCORES * NL, D), np.float32)
    for k in range(NCORES):
        out[k * NL:(k + 1) * NL] = res.results[k]["outT"].T
    return out[:n_real]


# revision 28
# speedup vs baseline: 1.0196x; 1.0196x over previous
"""CommNet GNN message-passing kernel for 8 Trainium2 NeuronCores.

Computation (matches the jax reference):
    h = relu(x @ enc_w1 + enc_b1) @ enc_w2 + enc_b2
    for r in range(R):
        msg[i] = mean over edges (src==i) of h[dst]
        h = h + relu(msg @ comm_w[r] + comm_b[r])
    out = relu(h @ dec_w1 + dec_b1) @ dec_w2 + dec_b2

Sharding: nodes (and their outgoing edges, partitioned by source node id)
are sharded across the 8 cores; MLP weights replicated. Each comm round
the per-core h shards are AllGather'd (bf16) into a full copy of h in
each core's DRAM; per-edge h[dst] reads are serviced by dma_gather
(256B bf16 rows).

The kernel is SWDGE-bound: descriptor generation for the gathers costs
~8ns/edge on the GpSimd Q7 pair and dominates everything else. This
version therefore:
  - consolidates gathers into per-group instructions (4 tiles/group,
    2 gathers each) to amortize the ~1.2us/instruction fixed cost;
  - keeps h_full in bf16 (halves gather DMA + AllGather bytes);
  - splits each AllGather into 4 chunked collectives issued as soon as
    their tiles' h rows are written, hiding the collective under the
    previous round's gathers (only the last chunk is exposed);
  - trims the tail of each gather via trailing -1 indices (the Q7 ucode
    skips them), with memzero covering the resulting stale blocks.

Per-core compute layout ("T layout": features on partitions, nodes on the
free axis) so every MLP matmul chains without transposes; segment-mean
via one-hot matmul into PSUM as before.
"""

import numpy as np

N = 50000
D = 128
R = 2
NCORES = 8
TILES = 49                # src-node tiles of 128 per core
NL = TILES * 128          # 6272 nodes per core
NP = NCORES * NL          # 50176 padded node count
SPLIT = 32768             # int16 gather-index limit -> lo/hi split of h rows
CHUNK_TILES = (32, 17)    # AllGather chunking; chunk 0 = exactly SPLIT rows
GROUP = 3                 # tiles per consolidated gather
TRIM = False              # trailing -1 idx trim (per-core DGE reduction)

_PROGRAM_CACHE: dict = {}

TRACE = False
LAST_RESULTS = None


def _chunks():
    """[(chunk_start_tile, n_tiles, row_offset_in_h_full), ...]"""
    out = []
    t0 = 0
    row = 0
    for ct in CHUNK_TILES:
        out.append((t0, ct, row))
        t0 += ct
        row += NCORES * ct * D
    assert t0 == TILES and row == NP
    return out


def _groups():
    """Gather groups (lists of tile ids), in PROCESSING order.

    Chunk-0 tiles go first: the next round's h_lo AllGather (issued after
    chunk 0's last tile, ~2/3 through the round) lands before the round
    ends, and the round prologue's two lo gathers (~130us) then cover the
    h_hi collective's mesh latency.
    """
    gs = []
    for (t0, ct, _row) in _chunks():
        t = t0
        while t < t0 + ct:
            g = list(range(t, min(t + GROUP, t0 + ct)))
            gs.append(g)
            t += GROUP
    return gs


# ----------------------------------------------------------------------------
# Device program
# ----------------------------------------------------------------------------

def build_program(layout):
    """Build the SPMD Bass program from the host-side layout dict."""
    import concourse.bass as bass
    import concourse.bacc as bacc
    import concourse.mybir as mybir
    import concourse.tile as tile

    dt = mybir.dt
    d = D
    nl = NL
    n_cores = NCORES
    tiles = TILES
    n_rounds = R

    groups = layout["groups"]      # list of tile lists
    lo_ba = layout["lo_ba"]        # per tile: first lo block in group section
    lo_bb = layout["lo_bb"]        # per tile: end lo block
    hi_ba = layout["hi_ba"]
    hi_bb = layout["hi_bb"]
    glo_nb = layout["glo_nb"]      # per group: total lo blocks
    ghi_nb = layout["ghi_nb"]      # per group: total hi blocks
    gilo_col = layout["gilo_col"]  # per group: col offset into ILO (int16 words)
    gihi_col = layout["gihi_col"]
    srcv_col = layout["srcv_col"]  # per tile: col offset into SRCV
    sum_b = layout["sum_b"]
    ilo_cols = layout["ilo_cols"]
    ihi_cols = layout["ihi_cols"]
    B_t = [(lo_bb[t] - lo_ba[t]) + (hi_bb[t] - hi_ba[t])
           for t in range(tiles)]
    gmax_nb = max(glo_nb[g] + ghi_nb[g] for g in range(len(groups)))
    bmax = max(B_t)

    nc = bacc.Bacc("TRN2", target_bir_lowering=False, debug=False,
                   num_devices=n_cores)

    # -------- kernel I/O --------
    xT_dram = nc.dram_tensor("xT", [d, nl], dt.float32, kind="ExternalInput")
    ilo_dram = nc.dram_tensor("ilo", [128, ilo_cols], dt.int16,
                              kind="ExternalInput")
    ihi_dram = nc.dram_tensor("ihi", [128, ihi_cols], dt.int16,
                              kind="ExternalInput")
    srcv_dram = nc.dram_tensor("srcv", [128, sum_b], dt.float32,
                               kind="ExternalInput")
    winv_dram = nc.dram_tensor("winv", [d, nl], dt.float32, kind="ExternalInput")
    iota_dram = nc.dram_tensor("iota", [d, d], dt.float32, kind="ExternalInput")
    ident_dram = nc.dram_tensor("ident", [d, d], dt.float32, kind="ExternalInput")
    w_drams = {}
    for wname in ("enc_w1", "enc_w2", "dec_w1", "dec_w2"):
        w_drams[wname] = nc.dram_tensor(wname, [d, d], dt.float32,
                                        kind="ExternalInput")
    for bname in ("enc_b1", "enc_b2", "dec_b1", "dec_b2"):
        w_drams[bname] = nc.dram_tensor(bname, [d, 1], dt.float32,
                                        kind="ExternalInput")
    for r in range(n_rounds):
        w_drams[f"cw{r}"] = nc.dram_tensor(f"cw{r}", [d, d], dt.float32,
                                           kind="ExternalInput")
        w_drams[f"cb{r}"] = nc.dram_tensor(f"cb{r}", [d, 1], dt.float32,
                                           kind="ExternalInput")
    outT_dram = nc.dram_tensor("outT", [d, nl], dt.float32,
                               kind="ExternalOutput")

    Relu = mybir.ActivationFunctionType.Relu
    Ident = mybir.ActivationFunctionType.Identity
    EQ = mybir.AluOpType.is_equal
    MUL = mybir.AluOpType.mult
    ADD = mybir.AluOpType.add

    with tile.TileContext(nc) as tc:
        with (
            tc.tile_pool(name="persist", bufs=1) as pp,
            tc.tile_pool(name="work", bufs=3) as wp,
            tc.tile_pool(name="gather", bufs=2) as gp,
            tc.tile_pool(name="ohpool", bufs=2) as op_,
            tc.tile_pool(name="ohfpool", bufs=1) as ofp,
            tc.tile_pool(name="psum", bufs=2, space="PSUM") as ps,
            tc.tile_pool(name="psum2", bufs=2, space="PSUM") as ps2,
            tc.tile_pool(name="dram", bufs=1, space="DRAM") as dp,
        ):
            # ---- persistent SBUF state ----
            xT = pp.tile([d, nl], dt.float32)
            hT = pp.tile([d, nl], dt.float32)
            winv = pp.tile([d, nl], dt.float32)
            iota = pp.tile([d, d], dt.float32)
            ident = pp.tile([d, d], dt.float32)
            wt = {}
            for wname in ("enc_w1", "enc_w2", "dec_w1", "dec_w2"):
                wt[wname] = pp.tile([d, d], dt.float32, tag=wname, name=wname)
            for bname in ("enc_b1", "enc_b2", "dec_b1", "dec_b2"):
                wt[bname] = pp.tile([d, 1], dt.float32, tag=bname, name=bname)
            for r in range(n_rounds):
                wt[f"cw{r}"] = pp.tile([d, d], dt.float32, tag=f"cw{r}",
                                       name=f"cw{r}")
                wt[f"cb{r}"] = pp.tile([d, 1], dt.float32, tag=f"cb{r}",
                                       name=f"cb{r}")

            nc.sync.dma_start(xT[:], xT_dram[:])
            nc.sync.dma_start(winv[:], winv_dram[:])
            nc.sync.dma_start(iota[:], iota_dram[:])
            nc.sync.dma_start(ident[:], ident_dram[:])
            for k, t in wt.items():
                nc.sync.dma_start(t[:], w_drams[k][:])

            # ---- DRAM scratch (bf16 h distribution) ----
            # Shared DRAM allows only one writer instruction per tensor, so
            # each AllGather chunk gets its own tensor; chunk 0 is exactly
            # the lo gather source, chunk 1 the hi source.
            ag_in = dp.tile([nl, d], dt.bfloat16)
            h_lo = [dp.tile([SPLIT, d], dt.bfloat16, addr_space="Shared",
                            tag=f"h_lo{r}", name=f"h_lo{r}")
                    for r in range(n_rounds)]
            h_hi = [dp.tile([NP - SPLIT, d], dt.bfloat16, addr_space="Shared",
                            tag=f"h_hi{r}", name=f"h_hi{r}")
                    for r in range(n_rounds)]

            def write_h_rows(t):
                """transpose hT[:, tile t] -> bf16 [node, feat] rows -> ag_in."""
                tsl = slice(t * d, (t + 1) * d)
                psT = ps2.tile([d, d], dt.float32, tag="psT")
                nc.tensor.transpose(psT[:], hT[:, tsl], ident[:])
                rowt = wp.tile([d, d], dt.bfloat16, tag="rowt")
                nc.scalar.copy(rowt[:], psT[:])
                nc.sync.dma_start(ag_in[tsl, :], rowt[:])

            def issue_ag(r, c):
                """AllGather chunk c of round r's h into h_lo/h_hi[r]."""
                t0, ct, _row = _chunks()[c]
                out = h_lo[r] if c == 0 else h_hi[r]
                nc.gpsimd.collective_compute(
                    "AllGather",
                    mybir.AluOpType.bypass,
                    replica_groups=[list(range(n_cores))],
                    ins=[ag_in[t0 * d:(t0 + ct) * d, :].opt()],
                    outs=[out.opt()],
                )

            # ================= encoder, chunked ======
            for c, (t0, ct, _row) in enumerate(_chunks()):
                t = t0
                while t < t0 + ct:
                    eg = min(4, t0 + ct - t)
                    gsl = slice(t * d, (t + eg) * d)
                    psA = ps.tile([d, 4 * d], dt.float32, tag="psA",
                                  padded_shape=[d, 4 * d])
                    nc.tensor.matmul(psA[:, 0:eg * d], wt["enc_w1"][:],
                                     xT[:, gsl], start=True, stop=True)
                    h1T = wp.tile([d, 4 * d], dt.float32, tag="h1T")
                    nc.scalar.activation(h1T[:, 0:eg * d], psA[:, 0:eg * d],
                                         Relu, bias=wt["enc_b1"][:])
                    psB = ps.tile([d, 4 * d], dt.float32, tag="psB",
                                  padded_shape=[d, 4 * d])
                    nc.tensor.matmul(psB[:, 0:eg * d], wt["enc_w2"][:],
                                     h1T[:, 0:eg * d], start=True, stop=True)
                    nc.scalar.activation(hT[:, gsl], psB[:, 0:eg * d], Ident,
                                         bias=wt["enc_b2"][:])
                    for tt in range(t, t + eg):
                        write_h_rows(tt)
                    t += eg
                issue_ag(0, c)

            # ================= comm rounds =================
            for r in range(n_rounds):
                src_lo = h_lo[r][:]
                src_hi = h_hi[r][:]
                chunk_done = {t0 + ct - 1: c for c, (t0, ct, _row)
                              in enumerate(_chunks())}

                gbufs = {}

                def alloc_gbuf(g):
                    gbufs[g] = gp.tile([128, gmax_nb, d], dt.bfloat16,
                                       tag="gbuf", name=f"gbuf_r{r}g{g}")

                def emit_lo(g):
                    n_lo = glo_nb[g] * 128
                    ilo = wp.tile([128, (gmax_nb * 128) // 16], dt.int16,
                                  tag="ilo")
                    nc.sync.dma_start(
                        ilo[:, 0:n_lo // 16],
                        ilo_dram[:, gilo_col[g]:gilo_col[g] + n_lo // 16])
                    nc.gpsimd.dma_gather(gbufs[g][:, 0:glo_nb[g], :], src_lo,
                                         ilo[:, 0:n_lo // 16], n_lo, n_lo,
                                         d, single_packet=False)

                def emit_hi(g):
                    lo_nb = glo_nb[g]
                    hi_nb = ghi_nb[g]
                    n_hi = hi_nb * 128
                    ihi = wp.tile([128, (gmax_nb * 128) // 16], dt.int16,
                                  tag="ihi")
                    nc.sync.dma_start(
                        ihi[:, 0:n_hi // 16],
                        ihi_dram[:, gihi_col[g]:gihi_col[g] + n_hi // 16])
                    nc.gpsimd.dma_gather(gbufs[g][:, lo_nb:lo_nb + hi_nb, :],
                                         src_hi, ihi[:, 0:n_hi // 16],
                                         n_hi, n_hi, d, single_packet=False)

                # The h_lo collective lands mid-previous-round; the two
                # frontloaded lo gathers (~130us) then cover the h_hi
                # collective's mesh latency at the round boundary.
                ng = len(groups)
                for g in range(min(2, ng)):
                    alloc_gbuf(g)
                    emit_lo(g)
                for g in range(min(2, ng)):
                    emit_hi(g)
                for g, gtiles in enumerate(groups):
                    lo_nb = glo_nb[g]
                    gbuf = gbufs[g]

                    for t in gtiles:
                        tsl = slice(t * d, (t + 1) * d)
                        bt = B_t[t]
                        # one-hot [128, bt*d]: oh[p, b*d + j] = (srcv[p,b] == j)
                        srcv = wp.tile([128, bmax], dt.float32, tag="srcv")
                        nc.sync.dma_start(
                            srcv[:, 0:bt],
                            srcv_dram[:, srcv_col[t]:srcv_col[t] + bt])
                        # DVE builds the one-hot in fp32 (a bf16 DVE output
                        # would engage 2-port mode and contend with the Q7's
                        # SBUF descriptor rings, stalling the gathers); the
                        # idle Scalar engine casts to bf16 for the matmul.
                        ohf = ofp.tile([128, bmax * d], dt.float32, tag="ohf")
                        oh = op_.tile([128, bmax * d], dt.bfloat16, tag="oh")
                        in0 = bass.AP(srcv.tensor, srcv.offset,
                                      [srcv.ap[0], [1, bt], [0, d]])
                        in1 = bass.AP(iota.tensor, iota.offset,
                                      [iota.ap[0], [0, bt], [1, d]])
                        out_oh = bass.AP(ohf.tensor, ohf.offset,
                                         [ohf.ap[0], [d, bt], [1, d]])
                        nc.vector.tensor_tensor(out_oh, in0, in1, EQ)
                        nc.scalar.copy(oh[:, 0:bt * d], ohf[:, 0:bt * d])
                        # segment sums: psM[f, n] += gathered_b.T @ onehot_b
                        psM = ps.tile([d, d], dt.float32, tag="psA")
                        nblk = 0
                        for b in range(lo_ba[t], lo_bb[t]):
                            nc.tensor.matmul(psM[:], gbuf[:, b, :],
                                             oh[:, nblk * d:(nblk + 1) * d],
                                             start=(nblk == 0),
                                             stop=(nblk == bt - 1))
                            nblk += 1
                        for b in range(hi_ba[t], hi_bb[t]):
                            nc.tensor.matmul(
                                psM[:], gbuf[:, lo_nb + b, :],
                                oh[:, nblk * d:(nblk + 1) * d],
                                start=(nblk == 0), stop=(nblk == bt - 1))
                            nblk += 1
                        # mean + comm MLP + residual
                        msgT = wp.tile([d, d], dt.float32, tag="msgT")
                        nc.vector.tensor_tensor(msgT[:], psM[:],
                                                winv[:, tsl], MUL)
                        psU = ps.tile([d, d], dt.float32, tag="psB")
                        nc.tensor.matmul(psU[:], wt[f"cw{r}"][:], msgT[:],
                                         start=True, stop=True)
                        updT = wp.tile([d, d], dt.float32, tag="updT")
                        nc.scalar.activation(updT[:], psU[:], Relu,
                                             bias=wt[f"cb{r}"][:])
                        nc.vector.tensor_tensor(hT[:, tsl], hT[:, tsl],
                                                updT[:], ADD)
                        if r + 1 < n_rounds:
                            write_h_rows(t)
                            if t in chunk_done:
                                issue_ag(r + 1, chunk_done[t])
                        else:
                            # final round: decoder fused per tile
                            psD = ps.tile([d, d], dt.float32, tag="psA",
                                          name="psD")
                            nc.tensor.matmul(psD[:], wt["dec_w1"][:],
                                             hT[:, tsl], start=True, stop=True)
                            d1T = wp.tile([d, d], dt.float32, tag="d1T")
                            nc.scalar.activation(d1T[:], psD[:], Relu,
                                                 bias=wt["dec_b1"][:])
                            psE = ps.tile([d, d], dt.float32, tag="psB",
                                          name="psE")
                            nc.tensor.matmul(psE[:], wt["dec_w2"][:], d1T[:],
                                             start=True, stop=True)
                            oT = wp.tile([d, d], dt.float32, tag="oT")
                            nc.scalar.activation(oT[:], psE[:], Ident,
                                                 bias=wt["dec_b2"][:])
                            nc.sync.dma_start(outT_dram[:, tsl], oT[:])

                    if g + 2 < ng:
                        alloc_gbuf(g + 2)
                        emit_lo(g + 2)
                        emit_hi(g + 2)

    nc.compile()
    return nc


# ----------------------------------------------------------------------------
# Host-side preparation
# ----------------------------------------------------------------------------

def _wrap_idx(idx):
    """int16 idx vector -> [128, n/16] layout: pos j -> (j%16, j//16), x8."""
    n = len(idx)
    a = np.zeros((16, n // 16), np.int16)
    a[np.arange(n) % 16, np.arange(n) // 16] = idx
    return np.tile(a, (8, 1))


def _row_remap():
    """node id (0..NP-1, old layout core-major) -> h_full row (chunk-major)."""
    remap = np.empty(NP, np.int64)
    n = np.arange(NP)
    k = n // NL
    t = (n % NL) // D
    p = n % D
    for (t0, ct, row) in _chunks():
        m = (t >= t0) & (t < t0 + ct)
        remap[n[m]] = (row + k[m] * ct * D + (t[m] - t0) * D + p[m])
    return remap


def host_prep(x, edge_index):
    """Shard + pad inputs; build per-core gather/one-hot side data."""
    d = D
    nl = NL
    n_real = x.shape[0]

    src = np.asarray(edge_index[0]).astype(np.int64)
    dst = np.asarray(edge_index[1]).astype(np.int64)

    cnt = np.bincount(src, minlength=NP).astype(np.float32)
    winv_full = 1.0 / np.maximum(cnt, 1.0)

    x_pad = np.zeros((NP, d), np.float32)
    x_pad[:n_real] = np.asarray(x, np.float32)

    remap = _row_remap()
    dstm = remap[dst]                # h_full row of each edge's dst

    # sort edges once by (tile, dst-row): tile-major grouping, ascending
    # dst row within a tile for friendlier gather locality
    tile_of_edge = src // d          # global tile id 0..n_cores*tiles-1
    order = np.lexsort((dstm, tile_of_edge))
    src_s, dstm_s = src[order], dstm[order]
    tile_s = tile_of_edge[order]
    lo_s = dstm_s < SPLIT

    n_tiles_g = NCORES * TILES
    tile_start = np.searchsorted(tile_s, np.arange(n_tiles_g))
    tile_end = np.searchsorted(tile_s, np.arange(n_tiles_g) + 1)
    n_lo_t = np.zeros(n_tiles_g, np.int64)
    for g in range(n_tiles_g):
        n_lo_t[g] = int(lo_s[tile_start[g]:tile_end[g]].sum())
    n_hi_t = (tile_end - tile_start) - n_lo_t

    # cross-core per-tile maxima (SPMD immediates) and minima (memzero range)
    core_ix = np.arange(NCORES) * TILES
    m_lo = [max(1, int(n_lo_t[core_ix + t].max())) for t in range(TILES)]
    m_hi = [max(1, int(n_hi_t[core_ix + t].max())) for t in range(TILES)]
    mn_lo = [int(n_lo_t[core_ix + t].min()) for t in range(TILES)]
    mn_hi = [int(n_hi_t[core_ix + t].min()) for t in range(TILES)]
    bl = [(m + d - 1) // d for m in m_lo]
    bh = [(m + d - 1) // d for m in m_hi]

    groups = _groups()
    # Unaligned slot packing: tile sections start at the running slot count
    # (not block-aligned); boundary blocks are shared between neighbouring
    # tiles and disambiguated by the -1 entries in each tile's srcv.
    lo_start = [0] * TILES    # slot offset of tile's lo section within group
    hi_start = [0] * TILES
    glo_nb, ghi_nb, gilo_col, gihi_col = [], [], [], []
    for g, gtiles in enumerate(groups):
        c = 0
        for t in gtiles:
            lo_start[t] = c
            c += m_lo[t]
        glo_nb.append((c + 127) // 128)
        c = 0
        for t in gtiles:
            hi_start[t] = c
            c += m_hi[t]
        ghi_nb.append((c + 127) // 128)
        gilo_col.append(sum(glo_nb[:g]) * 8)
        gihi_col.append(sum(ghi_nb[:g]) * 8)
    gilo_col = [0] * len(groups)
    gihi_col = [0] * len(groups)
    col_lo = col_hi = 0
    for g in range(len(groups)):
        gilo_col[g] = col_lo
        gihi_col[g] = col_hi
        col_lo += glo_nb[g] * 8
        col_hi += ghi_nb[g] * 8

    # per-tile block ranges [bA, bB) within the group's lo / hi sections
    lo_ba = [lo_start[t] // 128 for t in range(TILES)]
    lo_bb = [(lo_start[t] + m_lo[t] + 127) // 128 for t in range(TILES)]
    hi_ba = [hi_start[t] // 128 for t in range(TILES)]
    hi_bb = [(hi_start[t] + m_hi[t] + 127) // 128 for t in range(TILES)]

    srcv_col = [0] * TILES
    sb = 0
    for t in range(TILES):
        srcv_col[t] = sb
        sb += (lo_bb[t] - lo_ba[t]) + (hi_bb[t] - hi_ba[t])

    layout = {
        "m_lo": m_lo, "m_hi": m_hi, "groups": groups,
        "lo_start": lo_start, "hi_start": hi_start,
        "lo_ba": lo_ba, "lo_bb": lo_bb, "hi_ba": hi_ba, "hi_bb": hi_bb,
        "glo_nb": glo_nb, "ghi_nb": ghi_nb,
        "gilo_col": gilo_col, "gihi_col": gihi_col,
        "srcv_col": srcv_col,
        "sum_b": sb, "ilo_cols": col_lo, "ihi_cols": col_hi,
    }

    per_core = []
    for k in range(NCORES):
        ilo_all = np.zeros((128, col_lo), np.int16)
        ihi_all = np.zeros((128, col_hi), np.int16)
        srcv_all = np.full((128, sb), -1.0, np.float32)
        for g, gtiles in enumerate(groups):
            idx_lo = np.zeros(glo_nb[g] * 128, np.int16)
            idx_hi = np.zeros(ghi_nb[g] * 128, np.int16)
            for t in gtiles:
                gt = k * TILES + t
                s0, s1 = tile_start[gt], tile_end[gt]
                e_lo = np.flatnonzero(lo_s[s0:s1]) + s0
                e_hi = np.flatnonzero(~lo_s[s0:s1]) + s0
                idx_lo[lo_start[t]:lo_start[t] + len(e_lo)] = dstm_s[e_lo]
                idx_hi[hi_start[t]:hi_start[t] + len(e_hi)] = \
                    dstm_s[e_hi] - SPLIT
                # srcv covers the tile's block windows; -1 masks both the
                # pad slots and the neighbouring tiles' slots in shared
                # boundary blocks. slot s -> partition s%128, block s//128.
                nbl = lo_bb[t] - lo_ba[t]
                nbh = hi_bb[t] - hi_ba[t]
                bt = nbl + nbh
                slot_src = np.full(bt * 128, -1.0, np.float32)
                rel = lo_start[t] - lo_ba[t] * 128
                slot_src[rel:rel + len(e_lo)] = \
                    (src_s[e_lo] - gt * d).astype(np.float32)
                rel = nbl * 128 + (hi_start[t] - hi_ba[t] * 128)
                slot_src[rel:rel + len(e_hi)] = \
                    (src_s[e_hi] - gt * d).astype(np.float32)
                srcv_all[:, srcv_col[t]:srcv_col[t] + bt] = \
                    slot_src.reshape(bt, 128).T
            ilo_all[:, gilo_col[g]:gilo_col[g] + glo_nb[g] * 8] = \
                _wrap_idx(idx_lo)
            ihi_all[:, gihi_col[g]:gihi_col[g] + ghi_nb[g] * 8] = \
                _wrap_idx(idx_hi)
        ksl = slice(k * nl, (k + 1) * nl)
        per_core.append({
            "xT": np.ascontiguousarray(x_pad[ksl].T),
            "ilo": ilo_all,
            "ihi": ihi_all,
            "srcv": srcv_all,
            "winv": np.ascontiguousarray(
                np.tile(winv_full[ksl][None, :], (d, 1))),
        })
    return per_core, layout


def kernel(x, edge_index, enc_w1, enc_b1, enc_w2, enc_b2,
           comm_w, comm_b, dec_w1, dec_b1, dec_w2, dec_b2):
    from concourse.bass_utils import run_bass_kernel_spmd

    x = np.asarray(x)
    n_real = x.shape[0]
    per_core, layout = host_prep(x, np.asarray(edge_index))

    key = (tuple(layout["m_lo"]), tuple(layout["m_hi"]))
    if key not in _PROGRAM_CACHE:
        _PROGRAM_CACHE[key] = build_program(layout)
    nc = _PROGRAM_CACHE[key]

    iota_np = np.tile(np.arange(D, dtype=np.float32)[None, :], (D, 1))
    ident_np = np.eye(D, dtype=np.float32)
    shared = {
        "iota": iota_np,
        "ident": ident_np,
        "enc_w1": np.asarray(enc_w1, np.float32),
        "enc_w2": np.asarray(enc_w2, np.float32),
        "dec_w1": np.asarray(dec_w1, np.float32),
        "dec_w2": np.asarray(dec_w2, np.float32),
        "enc_b1": np.asarray(enc_b1, np.float32).reshape(D, 1),
        "enc_b2": np.asarray(enc_b2, np.float32).reshape(D, 1),
        "dec_b1": np.asarray(dec_b1, np.float32).reshape(D, 1),
        "dec_b2": np.asarray(dec_b2, np.float32).reshape(D, 1),
    }
    for r in range(R):
        shared[f"cw{r}"] = np.asarray(comm_w[r], np.float32)
        shared[f"cb{r}"] = np.asarray(comm_b[r], np.float32).reshape(D, 1)

    in_maps = [{**shared, **pc} for pc in per_core]
    res = run_bass_kernel_spmd(nc, in_maps, core_ids=list(range(NCORES)),
                               trace=TRACE)
    global LAST_RESULTS
    LAST_RESULTS = res

    out = np.empty((NCORES * NL, D), np.float32)
    for k in range(NCORES):
        out[k * NL:(k + 1) * NL] = res.results[k]["outT"].T
    return out[:n_real]
